# revision 38
# baseline (speedup 1.0000x reference)
"""ChunkGNNEncoder Trainium kernel: host prep + Bass/Tile kernel builder.

Math (per GCN layer, PyG GCNConv):
  h = x @ W              (dense, per-core nodes)
  g = dinv * h           (dinv = deg^-1/2, deg = in-degree incl self-loop)
  agg[t] = g[t] + sum_{edges (s,t)} g[s]      (self-loop added locally)
  h' = relu(dinv[t] * agg[t] + b)
Then global mean pool per graph, final linear.

Device decomposition (8 cores):
  - nodes sharded by graph (8 graphs/core), relabeled into NW windows of 128
  - dense matmuls local; g slices AllGather'd into per-segment tables (int16
    gather index limit -> NSEG=2 segments, each table <= 32767 rows)
  - aggregation: dma_gather rows from table + one-hot matmul scatter into
    PSUM window accumulators (S^T built by DVE is_equal vs iota); gathers
    round-robin across 4 SWDGE queues so descriptor-ring stalls overlap
  - self-loop term via identity-matmul PSUM init from SBUF-resident local g
    (keeps self-loops out of the gather: ~12% fewer gathered rows)
  - bias+relu on DVE at flush (b broadcast tile), no PE outer-product init
  - flush: h' = relu(dinv*psum + b); PE-transpose feeds next dense matmul
  - mean-pool via 0/1 selector matmul accumulated over windows
"""

import numpy as np
import ml_dtypes
from dataclasses import dataclass, field

import concourse.bass as bass
import concourse.bacc as bacc
import concourse.mybir as mybir
import concourse.tile as tile


@dataclass
class Cfg:
    n_nodes: int = 50000
    n_edges: int = 300000
    n_graphs: int = 64
    in_dim: int = 768
    hid: int = 256
    out_dim: int = 128
    n_cores: int = 8
    nw: int = 52          # windows per core (128 nodes each)
    nseg: int = 2         # source segments (int16 tables); nw % nseg == 0
    rw: int = 4           # windows per aggregation range (PSUM-resident)

    @property
    def p_local(self):
        return self.nw * 128

    @property
    def wseg(self):
        return self.nw // self.nseg

    @property
    def segrows(self):
        return self.wseg * 128

    @property
    def gpc(self):
        return self.n_graphs // self.n_cores

    @property
    def kin(self):
        return self.in_dim // 128

    @property
    def khid(self):
        return self.hid // 128

    @property
    def ranges(self):
        r = []
        w = 0
        while w < self.nw:
            r.append((w, min(w + self.rw, self.nw)))
            w += self.rw
        return r


@dataclass
class Meta:
    # chunks per (seg, window): baked max over cores
    C: np.ndarray = None          # [nseg, nw] int
    # per (range, seg): start chunk index into global chunk order, n chunks
    batch_chunk_base: dict = field(default_factory=dict)
    tot_chunks: int = 0
    cbmax: int = 0                # max chunks in one (range, seg) batch


def host_prep(x, edge_index, batch, W1, b1, W2, b2, Wp, bp, cfg: Cfg):
    """Returns (in_maps: list of per-core dict, meta: Meta)."""
    N, E, G = cfg.n_nodes, cfg.n_edges, cfg.n_graphs
    NC, NW, NSEG = cfg.n_cores, cfg.nw, cfg.nseg
    src = np.asarray(edge_index[0], dtype=np.int64)
    tgt = np.asarray(edge_index[1], dtype=np.int64)
    batch = np.asarray(batch, dtype=np.int64)

    deg = np.bincount(tgt, minlength=N).astype(np.float64) + 1.0
    dinv = (1.0 / np.sqrt(deg)).astype(np.float32)

    node_core = batch // cfg.gpc                      # graph g -> core g//gpc
    # per-core node lists (batch is sorted, nodes contiguous per core)
    core_nodes = [np.nonzero(node_core == c)[0] for c in range(NC)]
    for c in range(NC):
        assert len(core_nodes[c]) <= cfg.p_local, (
            f"core {c} has {len(core_nodes[c])} nodes > {cfg.p_local}; raise nw")

    # in-degree per node (incl self)
    indeg = np.bincount(tgt, minlength=N) + 1

    # --- window bin-packing per core: balance in-degree, <=128 nodes/window
    local_row = np.full(N, -1, np.int64)      # node -> local row on its core
    for c in range(NC):
        nodes = core_nodes[c]
        order = np.argsort(-indeg[nodes], kind="stable")
        wload = np.zeros(NW, np.int64)
        wcount = np.zeros(NW, np.int64)
        slot_of = np.empty(len(nodes), np.int64)
        # greedy: put next-heaviest node into least-loaded non-full window
        for i in order:
            open_w = np.nonzero(wcount < 128)[0]
            w = open_w[np.argmin(wload[open_w])]
            slot_of[i] = w * 128 + wcount[w]
            wcount[w] += 1
            wload[w] += indeg[nodes[i]]
        local_row[nodes] = slot_of

    # --- edge lists (NO self-loops: handled by identity matmul on-device),
    # bucketed by (target core, seg, window)
    e_core = node_core[tgt]
    e_lrow_t = local_row[tgt]
    e_w = e_lrow_t // 128
    e_tshift = e_lrow_t % 128
    s_core = node_core[src]
    s_lrow = local_row[src]
    e_seg = s_lrow // cfg.segrows
    # index into segment table: rank-major layout (AllGather concat on rows)
    e_idx16 = s_core * cfg.segrows + (s_lrow - e_seg * cfg.segrows)
    assert e_idx16.max() < 32768

    # counts per (core, seg, window)
    cnt = np.zeros((NC, NSEG, NW), np.int64)
    np.add.at(cnt, (e_core, e_seg, e_w), 1)
    C = np.maximum(0, -(-cnt.max(axis=0) // 128))     # [NSEG, NW]

    meta = Meta()
    meta.C = C

    # global chunk order: range -> seg -> window -> chunks
    chunk_order = []      # list of (seg, window)-chunk tuples in order
    batch_base = {}
    for (w0, w1) in cfg.ranges:
        for q in range(NSEG):
            base = len(chunk_order)
            for w in range(w0, w1):
                for _ in range(C[q, w]):
                    chunk_order.append((q, w))
            batch_base[(w0, q)] = (base, len(chunk_order) - base)
    meta.batch_chunk_base = batch_base
    meta.tot_chunks = len(chunk_order)
    meta.cbmax = max((n for (_, n) in batch_base.values()), default=1)
    TOTC = meta.tot_chunks
    TOTS = TOTC * 128

    # slot ranges per (seg,window) in the global order
    slot_base = {}
    pos = 0
    for (w0, w1) in cfg.ranges:
        for q in range(NSEG):
            for w in range(w0, w1):
                slot_base[(q, w)] = pos
                pos += C[q, w] * 128
    assert pos == TOTS

    # --- per-core edge slot arrays
    dt_bf16 = ml_dtypes.bfloat16
    dt_fp8 = ml_dtypes.float8_e4m3
    in_maps = []
    W1b = np.asarray(W1, np.float32).astype(dt_bf16)
    W2b = np.asarray(W2, np.float32).astype(dt_bf16)
    Wpb = np.asarray(Wp, np.float32).astype(dt_bf16)
    b1f = np.asarray(b1, np.float32).reshape(1, -1)
    b2f = np.asarray(b2, np.float32).reshape(1, -1)
    bpf = np.asarray(bp, np.float32).reshape(1, -1)
    b1bc = np.tile(b1f, (128, 1))                     # [128, HID]
    b2bc = np.tile(b2f, (128, 1))
    x = np.asarray(x, np.float32)

    iota_rep = np.tile(np.arange(128, dtype=np.int8)[None, :],
                       (128, max(meta.cbmax, 1)))

    for c in range(NC):
        mask = e_core == c
        cs, cw, ct, cq = (e_seg[mask], e_w[mask], e_tshift[mask],
                          e_idx16[mask])
        idx_flat = np.zeros(TOTS, np.int64)
        tsh_flat = np.full(TOTS, -1, np.int64)
        # fill per (q,w)
        key = cs * NW + cw
        order = np.argsort(key, kind="stable")
        ks, kt, kq, kw = cq[order], ct[order], cs[order], cw[order]
        # walk groups
        uniq, starts = np.unique(kq * NW + kw, return_index=True)
        starts = list(starts) + [len(ks)]
        for u, s0, s1 in zip(uniq, starts[:-1], starts[1:]):
            q, w = int(u) // NW, int(u) % NW
            n = s1 - s0
            b = slot_base[(q, w)]
            assert n <= C[q, w] * 128
            so = np.argsort(ks[s0:s1], kind="stable")
            idx_flat[b:b + n] = ks[s0:s1][so]
            tsh_flat[b:b + n] = kt[s0:s1][so]

        # pack idx: [128, TOTS/16] int16, slot j -> [16r + j%16, j//16]
        idxp = idx_flat.reshape(-1, 16).T.astype(np.int16)       # [16, TOTS/16]
        idxp = np.tile(idxp, (8, 1))                              # [128, ...]
        # tshift: [128, TOTC] int8: chunk k slot p -> [p, k]
        tshp = tsh_flat.reshape(TOTC, 128).T.astype(np.int8)

        # xT bf16 [in_dim, p_local]
        nodes = core_nodes[c]
        xT = np.zeros((cfg.in_dim, cfg.p_local), np.float32)
        xT[:, local_row[nodes]] = x[nodes].T
        xTb = xT.astype(dt_bf16)

        # dinv cols [128, NW]
        dinv_l = np.zeros(cfg.p_local, np.float32)
        dinv_l[local_row[nodes]] = dinv[nodes]
        dinv_cols = dinv_l.reshape(NW, 128).T.copy()              # [128, NW]

        # pool selector [128, NW*gpc] bf16 and cntinv [128, khid*gpc]
        spool = np.zeros((cfg.p_local, cfg.gpc), np.float32)
        gl = batch[nodes] - c * cfg.gpc
        spool[local_row[nodes], gl] = 1.0
        spool = (spool.reshape(NW, 128, cfg.gpc).transpose(1, 0, 2)
                 .reshape(128, NW * cfg.gpc)).astype(dt_bf16)
        cnt_g = np.bincount(gl, minlength=cfg.gpc).astype(np.float32)
        cntinv = (1.0 / np.maximum(cnt_g, 1.0)).astype(np.float32)
        cntinv_rep = np.tile(cntinv[None, :], (128, cfg.khid))    # [128, khid*gpc]

        ident = np.eye(128, dtype=dt_bf16)
        identq = np.eye(128, dtype=dt_fp8)

        in_maps.append(dict(
            xT=xTb, W1=W1b, W2=W2b, Wp=Wpb,
            b1b=b1bc, b2b=b2bc,
            bp8=np.tile(bpf, (cfg.gpc, 1)).astype(np.float32),
            dinv_cols=dinv_cols,
            idx=idxp, tsh=tshp, iota=iota_rep, spool=spool,
            cntinv=cntinv_rep, ident=ident, identq=identq,
        ))
    return in_maps, meta


def build_kernel(cfg: Cfg, meta: Meta, debug=False):
    NC, NW, NSEG = cfg.n_cores, cfg.nw, cfg.nseg
    HID, OUT, GPC = cfg.hid, cfg.out_dim, cfg.gpc
    KIN, KHID = cfg.kin, cfg.khid
    C = meta.C
    TOTC = meta.tot_chunks
    TOTS = TOTC * 128
    bf16, f32 = mybir.dt.bfloat16, mybir.dt.float32
    fp8 = mybir.dt.float8e4
    tabrows = NC * cfg.segrows
    WSEG = cfg.wseg

    nc = bacc.Bacc(None, target_bir_lowering=False, debug=debug,
                   num_devices=NC if NC > 1 else None,
                   num_swdge_queues=4,
                   dynamic_dma_scratch_size=32768)

    dram_in = lambda n, s, d: nc.dram_tensor(n, s, d, kind="ExternalInput")
    xT_d = dram_in("xT", [cfg.in_dim, cfg.p_local], bf16)
    W1_d = dram_in("W1", [cfg.in_dim, HID], bf16)
    W2_d = dram_in("W2", [HID, HID], bf16)
    Wp_d = dram_in("Wp", [HID, OUT], bf16)
    b1b_d = dram_in("b1b", [128, HID], f32)
    b2b_d = dram_in("b2b", [128, HID], f32)
    bp8_d = dram_in("bp8", [GPC, OUT], f32)
    dinv_d = dram_in("dinv_cols", [128, NW], f32)
    idx_d = dram_in("idx", [128, TOTS // 16], mybir.dt.int16)
    tsh_d = dram_in("tsh", [128, TOTC], mybir.dt.int8)
    iota_d = dram_in("iota", [128, 128 * meta.cbmax], mybir.dt.int8)
    spool_d = dram_in("spool", [128, NW * GPC], bf16)
    cntinv_d = dram_in("cntinv", [128, KHID * GPC], f32)
    ident_d = dram_in("ident", [128, 128], bf16)
    identq_d = dram_in("identq", [128, 128], fp8)
    out_d = nc.dram_tensor("out", [GPC, OUT], f32, kind="ExternalOutput")

    with tile.TileContext(nc) as tc:
        with (
            tc.tile_pool(name="const", bufs=1) as cpool,
            tc.tile_pool(name="xw", bufs=3) as xwpool,
            tc.tile_pool(name="gbuf", bufs=12) as gpool,
            tc.tile_pool(name="stb", bufs=6) as stpool,
            tc.tile_pool(name="flush", bufs=4) as fpool,
            tc.tile_pool(name="gall", bufs=1) as gallpool,
            tc.tile_pool(name="psw", bufs=cfg.rw + 1, space="PSUM") as pswin,
            tc.tile_pool(name="psx", bufs=2, space="PSUM") as psx,
            tc.tile_pool(name="pspool", bufs=1, space="PSUM") as pspool,
            tc.tile_pool(name="dram", bufs=1, space="DRAM") as dram,
        ):
            # ---- constants to SBUF
            W1_t = cpool.tile([128, KIN, HID], bf16)
            nc.sync.dma_start(W1_t[:], W1_d[:].rearrange("(k p) n -> p k n", p=128))
            W2_t = cpool.tile([128, KHID, HID], bf16)
            nc.sync.dma_start(W2_t[:], W2_d[:].rearrange("(k p) n -> p k n", p=128))
            Wp_t = cpool.tile([128, KHID, OUT], bf16)
            nc.sync.dma_start(Wp_t[:], Wp_d[:].rearrange("(k p) n -> p k n", p=128))
            b1b_t = cpool.tile([128, HID], f32)
            nc.sync.dma_start(b1b_t[:], b1b_d[:])
            b2b_t = cpool.tile([128, HID], f32)
            nc.sync.dma_start(b2b_t[:], b2b_d[:])
            bp8_t = cpool.tile([GPC, OUT], f32)
            nc.sync.dma_start(bp8_t[:], bp8_d[:])
            dinv_t = cpool.tile([128, NW], f32)
            nc.sync.dma_start(dinv_t[:], dinv_d[:])
            idx_t = cpool.tile([128, TOTS // 16], mybir.dt.int16)
            nc.sync.dma_start(idx_t[:], idx_d[:])
            tsh_t = cpool.tile([128, TOTC], mybir.dt.int8)
            nc.sync.dma_start(tsh_t[:], tsh_d[:])
            iota_t = cpool.tile([128, 128 * meta.cbmax], mybir.dt.int8)
            nc.sync.dma_start(iota_t[:], iota_d[:])
            spool_t = cpool.tile([128, NW * GPC], bf16)
            nc.sync.dma_start(spool_t[:], spool_d[:])
            cntinv_t = cpool.tile([128, KHID * GPC], f32)
            nc.sync.dma_start(cntinv_t[:], cntinv_d[:])
            ident_t = cpool.tile([128, 128], bf16)
            nc.sync.dma_start(ident_t[:], ident_d[:])
            identq_t = cpool.tile([128, 128], fp8)
            nc.sync.dma_start(identq_t[:], identq_d[:])
            zrow_t = cpool.tile([1, KHID * GPC], bf16)
            nc.vector.memset(zrow_t[:], 0.0)

            # ---- AG tables (DRAM), fp8 message rows (CoreSim requires a
            # single collective writer per Shared tile -> one AG per seg)
            ag_in = [[dram.tile([cfg.segrows, HID], fp8, tag=f"agin{l}{q}",
                                name=f"agin{l}{q}")
                      for q in range(NSEG)] for l in range(2)]
            ag_out = [[dram.tile([tabrows, HID], fp8, tag=f"agout{l}{q}",
                                 name=f"agout{l}{q}",
                                 addr_space="Shared" if NC > 1 else "Local")
                       for q in range(NSEG)] for l in range(2)]

            # local g per layer, SBUF-resident (self-loop source + AG staging)
            g_sb = [gallpool.tile([128, NW, HID], fp8, tag=f"gsb{l}",
                                  name=f"gsb{l}") for l in range(2)]

            xt2_t = gallpool.tile([128, KHID, cfg.p_local], bf16, tag="xt2",
                                  name="xt2")

            def stage_seg(layer, q):
                """DMA g seg q -> ag_in on the Sync engine (idle during agg,
                so the transfer fires as soon as the g windows are written
                instead of queueing behind ACT flush ops)."""
                dst = (ag_out if NC == 1 else ag_in)[layer][q]
                nc.sync.dma_start(
                    dst[:].rearrange("(w p) n -> p w n", p=128),
                    g_sb[layer][:, q * WSEG:(q + 1) * WSEG, :])

            def launch_ag(layer, q):
                if NC == 1:
                    return
                nc.gpsimd.collective_compute(
                    "AllGather", mybir.AluOpType.bypass,
                    replica_groups=[list(range(NC))],
                    ins=[ag_in[layer][q][:].opt()],
                    outs=[ag_out[layer][q][:].opt()],
                )

            def stage_and_ag(layer, q):
                stage_seg(layer, q)
                launch_ag(layer, q)

            # ================= dense layer 1 (segment-ordered windows) ======
            for w in range(NW):
                xw = xwpool.tile([128, KIN, 128], bf16, tag="xw")
                nc.sync.dma_start(
                    xw[:],
                    xT_d[:, w * 128:(w + 1) * 128]
                    .rearrange("(k p) n -> p k n", p=128))
                ps = psx.tile([128, HID], f32, tag="psx")
                for k in range(KIN):
                    nc.tensor.matmul(ps[:], xw[:, k, :], W1_t[:, k, :],
                                     start=(k == 0), stop=(k == KIN - 1))
                # g = dinv*h, cast to fp8 table row (ACT engine, idle)
                nc.scalar.activation(g_sb[0][:, w, :], ps[:],
                                     mybir.ActivationFunctionType.Copy,
                                     scale=dinv_t[:, w:w + 1])
                if (w + 1) % WSEG == 0:
                    stage_and_ag(0, w // WSEG)

            # ================= aggregation + layer-2 dense (fused) ==========
            gq_counter = [0]   # SWDGE queue round-robin across gather calls

            def agg_layer(layer):
                """layer 0: consume ag_out[0], produce h1' -> xt2 + g2 (+AG)
                   layer 1: consume ag_out[1], produce pooled psum"""
                if layer == 1:
                    pooled = pspool.tile([128, KHID * GPC], f32)
                    # single zeroing init for the whole pooled bank: later
                    # pooling matmuls all accumulate (start would wipe the
                    # full 2KB zero region, clobbering the sibling group)
                    nc.tensor.matmul(pooled[:], ident_t[0:1, :],
                                     zrow_t[:], start=True, stop=False,
                                     skip_group_check=True)
                bias_t = b1b_t if layer == 0 else b2b_t
                # deferred AG launches (layer 0 only): seg -> emit-after range
                pending_ag = {}
                for ri, (w0, w1) in enumerate(cfg.ranges):
                    nwin = w1 - w0
                    # PSUM is bank-granular (2KB = 512 f32 cols): pack TWO
                    # windows per bank so 2.5 ranges fit in flight. The first
                    # ident matmul's start=True zeroes the whole bank; the
                    # sibling window inits with start=False onto the zeros.
                    nbank = (nwin + 1) // 2
                    pbs = [pswin.tile([128, 2, HID], f32, tag="aggpsum",
                                      name=f"aggpsum_{layer}_{w0}_{j}")
                           for j in range(nbank)]
                    # chunks remaining per bank (both windows, all segs)
                    rem = [sum(int(C[:, w].sum())
                               for w in range(w0 + 2 * j,
                                              min(w0 + 2 * j + 2, w1)))
                           for j in range(nbank)]
                    # self-loop init: psum_w = I^T @ g_w  (exact: dinv[t]*g[t]
                    # = dinv^2*h self term after flush scaling)
                    for i in range(nwin):
                        w = w0 + i
                        j, half = i // 2, i % 2
                        last_of_bank = (rem[j] == 0 and
                                        (half == 1 or w + 1 == w1))
                        nc.tensor.matmul(
                            pbs[j][:, half, :], identq_t[:],
                            g_sb[layer][:, w, :],
                            start=(half == 0), stop=last_of_bank,
                            skip_group_check=True)
                    for q in range(NSEG):
                        base, nchk = meta.batch_chunk_base[(w0, q)]
                        if nchk == 0:
                            continue
                        gb = gpool.tile([128, meta.cbmax, HID], fp8, tag="gb")
                        # SWDGE descriptor ring caps one dma_gather at ~1024
                        # indices; split into <=GMAX-chunk pieces round-robined
                        # over 4 SWDGE queues so ring-space stalls overlap
                        GMAX = 8
                        for g0 in range(0, nchk, GMAX):
                            g1 = min(g0 + GMAX, nchk)
                            nc.gpsimd.dma_gather(
                                gb[:, g0:g1, :], ag_out[layer][q][:],
                                idx_t[:, (base + g0) * 8:(base + g1) * 8],
                                num_idxs=(g1 - g0) * 128,
                                num_idxs_reg=(g1 - g0) * 128,
                                elem_size=HID,
                                queue_num=gq_counter[0] % 4)
                            gq_counter[0] += 1
                        st = stpool.tile([128, meta.cbmax * 128], fp8, tag="st")
                        nc.vector.tensor_tensor(
                            out=st[:, :nchk * 128].rearrange(
                                "p (c i) -> p c i", i=128),
                            in0=tsh_t[:, base:base + nchk].unsqueeze(2)
                                .broadcast_to([128, nchk, 128]),
                            in1=iota_t[:, :nchk * 128].rearrange(
                                "p (c i) -> p c i", i=128),
                            op=mybir.AluOpType.is_equal)
                        ci = 0
                        for w in range(w0, w1):
                            j, half = (w - w0) // 2, (w - w0) % 2
                            for _ in range(C[q, w]):
                                rem[j] -= 1
                                nc.tensor.matmul(
                                    pbs[j][:, half, :],
                                    st[:, ci * 128:(ci + 1) * 128],
                                    gb[:, ci, :],
                                    start=False, stop=(rem[j] == 0),
                                    skip_group_check=True)
                                ci += 1
                        assert ci == nchk
                    # flush windows of this range
                    for i, w in enumerate(range(w0, w1)):
                        # t1 = dinv*psum on ACT (Copy w/ per-partition scale;
                        # Copy-only keeps the ACT table warm)
                        tmp = fpool.tile([128, HID], f32, tag="tmpf")
                        nc.scalar.activation(tmp[:], pbs[i // 2][:, i % 2, :],
                                             mybir.ActivationFunctionType.Copy,
                                             scale=dinv_t[:, w:w + 1])
                        hp0 = fpool.tile([128, HID], bf16, tag="hpre")
                        nc.vector.tensor_tensor(out=hp0[:], in0=tmp[:],
                                                in1=bias_t[:],
                                                op=mybir.AluOpType.add)
                        # relu on ACT: Relu is a filler function in every
                        # table set, so no ACT_TABLE_LOAD thrash vs Copy
                        hp = fpool.tile([128, HID], bf16, tag="hflush")
                        nc.scalar.activation(hp[:], hp0[:],
                                             mybir.ActivationFunctionType.Relu)
                        if layer == 0:
                            # transpose into xt2; dense2; g2 write
                            for h in range(KHID):
                                pt = psx.tile([128, 128], bf16, tag="psx")
                                nc.tensor.transpose(
                                    pt[:], hp[:, h * 128:(h + 1) * 128],
                                    ident_t[:])
                                nc.vector.tensor_copy(
                                    xt2_t[:, h, w * 128:(w + 1) * 128], pt[:])
                            ps2 = psx.tile([128, HID], f32, tag="psx")
                            for k in range(KHID):
                                nc.tensor.matmul(
                                    ps2[:], xt2_t[:, k, w * 128:(w + 1) * 128],
                                    W2_t[:, k, :],
                                    start=(k == 0), stop=(k == KHID - 1))
                            nc.scalar.activation(
                                g_sb[1][:, w, :], ps2[:],
                                mybir.ActivationFunctionType.Copy,
                                scale=dinv_t[:, w:w + 1])
                            if (w + 1) % WSEG == 0:
                                # stage now (sync engine fires on data-ready);
                                # defer the collective ~2 ranges so it doesn't
                                # head-of-line block pending gather calls
                                stage_seg(1, w // WSEG)
                                pending_ag[w // WSEG] = ri + 2
                        else:
                            for h in range(KHID):
                                nc.tensor.matmul(
                                    pooled[:, h * GPC:(h + 1) * GPC],
                                    hp[:, h * 128:(h + 1) * 128],
                                    spool_t[:, w * GPC:(w + 1) * GPC],
                                    start=False,
                                    stop=(w == NW - 1 and h == KHID - 1),
                                    skip_group_check=True)
                    # emit any AG whose deferral window has elapsed
                    for q in list(pending_ag):
                        if ri >= pending_ag[q] or ri == len(cfg.ranges) - 1:
                            launch_ag(1, q)
                            del pending_ag[q]
                if layer == 1:
                    return pooled

            agg_layer(0)
            pooled = agg_layer(1)
            # ============= pooled -> mean -> final linear ===============
            pooledT = fpool.tile([128, KHID * GPC], bf16, tag="pooledT")
            nc.vector.tensor_tensor(out=pooledT[:], in0=pooled[:],
                                    in1=cntinv_t[:],
                                    op=mybir.AluOpType.mult)
            ps_out = psx.tile([GPC, OUT], f32, tag="psx")
            for k in range(KHID):
                nc.tensor.matmul(ps_out[:],
                                 pooledT[:, k * GPC:(k + 1) * GPC],
                                 Wp_t[:, k, :],
                                 start=(k == 0), stop=(k == KHID - 1))
            out_sb = fpool.tile([GPC, OUT], f32, tag="outsb")
            nc.vector.tensor_tensor(out=out_sb[:], in0=ps_out[:],
                                    in1=bp8_t[:], op=mybir.AluOpType.add)
            nc.sync.dma_start(out_d[:], out_sb[:])

    nc.compile()
    return nc


def kernel(**inputs) -> "np.ndarray":
    """Full-input entry point: shards the graph across 8 NeuronCores,
    runs the Bass GNN kernel, returns the full (64, 128) float32 output."""
    cfg = Cfg()
    in_maps, meta = host_prep(
        inputs["x"], inputs["edge_index"], inputs["batch"],
        inputs["W1"], inputs["b1"], inputs["W2"], inputs["b2"],
        inputs["Wp"], inputs["bp"], cfg)
    nc = build_kernel(cfg, meta, debug=False)
    from concourse.bass_utils import run_bass_kernel_spmd
    res = run_bass_kernel_spmd(nc, in_maps,
                               core_ids=list(range(cfg.n_cores)), trace=False)
    out = np.concatenate([r["out"] for r in res.results], axis=0)
    return np.ascontiguousarray(out.astype(np.float32))


# revision 42
# speedup vs baseline: 1.0739x; 1.0739x over previous
"""ChunkGNNEncoder Trainium kernel: host prep + Bass/Tile kernel builder.

Math (per GCN layer, PyG GCNConv):
  h = x @ W              (dense, per-core nodes)
  g = dinv * h           (dinv = deg^-1/2, deg = in-degree incl self-loop)
  agg[t] = g[t] + sum_{edges (s,t)} g[s]      (self-loop added locally)
  h' = relu(dinv[t] * agg[t] + b)
Then global mean pool per graph, final linear.

Device decomposition (8 cores):
  - nodes sharded by graph (8 graphs/core), relabeled into NW windows of 128
  - dense matmuls local; g slices AllGather'd into per-segment tables (int16
    gather index limit -> NSEG=2 segments, each table <= 32767 rows)
  - aggregation: dma_gather rows from table + one-hot matmul scatter into
    PSUM window accumulators (S^T built by DVE is_equal vs iota); gathers
    round-robin across 4 SWDGE queues so descriptor-ring stalls overlap
  - self-loop term via identity-matmul PSUM init from SBUF-resident local g
    (keeps self-loops out of the gather: ~12% fewer gathered rows)
  - bias+relu on DVE at flush (b broadcast tile), no PE outer-product init
  - flush: h' = relu(dinv*psum + b); PE-transpose feeds next dense matmul
  - mean-pool via 0/1 selector matmul accumulated over windows
"""

import numpy as np
import ml_dtypes
from dataclasses import dataclass, field

import concourse.bass as bass
import concourse.bacc as bacc
import concourse.mybir as mybir
import concourse.tile as tile


@dataclass
class Cfg:
    n_nodes: int = 50000
    n_edges: int = 300000
    n_graphs: int = 64
    in_dim: int = 768
    hid: int = 256
    out_dim: int = 128
    n_cores: int = 8
    nw: int = 52          # windows per core (128 nodes each)
    nseg: int = 2         # source segments (int16 tables); nw % nseg == 0
    rw: int = 4           # windows per aggregation range (PSUM-resident)

    @property
    def p_local(self):
        return self.nw * 128

    @property
    def wseg(self):
        return self.nw // self.nseg

    @property
    def segrows(self):
        return self.wseg * 128

    @property
    def gpc(self):
        return self.n_graphs // self.n_cores

    @property
    def kin(self):
        return self.in_dim // 128

    @property
    def khid(self):
        return self.hid // 128

    @property
    def ranges(self):
        r = []
        w = 0
        while w < self.nw:
            r.append((w, min(w + self.rw, self.nw)))
            w += self.rw
        return r


@dataclass
class Meta:
    # chunks per (seg, window): baked max over cores
    C: np.ndarray = None          # [nseg, nw] int
    # per (range, seg): start chunk index into global chunk order, n chunks
    batch_chunk_base: dict = field(default_factory=dict)
    tot_chunks: int = 0
    cbmax: int = 0                # max chunks in one (range, seg) batch


def host_prep(x, edge_index, batch, W1, b1, W2, b2, Wp, bp, cfg: Cfg):
    """Returns (in_maps: list of per-core dict, meta: Meta)."""
    N, E, G = cfg.n_nodes, cfg.n_edges, cfg.n_graphs
    NC, NW, NSEG = cfg.n_cores, cfg.nw, cfg.nseg
    src = np.asarray(edge_index[0], dtype=np.int64)
    tgt = np.asarray(edge_index[1], dtype=np.int64)
    batch = np.asarray(batch, dtype=np.int64)

    deg = np.bincount(tgt, minlength=N).astype(np.float64) + 1.0
    dinv = (1.0 / np.sqrt(deg)).astype(np.float32)

    node_core = batch // cfg.gpc                      # graph g -> core g//gpc
    # per-core node lists (batch is sorted, nodes contiguous per core)
    core_nodes = [np.nonzero(node_core == c)[0] for c in range(NC)]
    for c in range(NC):
        assert len(core_nodes[c]) <= cfg.p_local, (
            f"core {c} has {len(core_nodes[c])} nodes > {cfg.p_local}; raise nw")

    # in-degree per node (incl self)
    indeg = np.bincount(tgt, minlength=N) + 1

    # --- window bin-packing per core: balance in-degree, <=128 nodes/window
    local_row = np.full(N, -1, np.int64)      # node -> local row on its core
    for c in range(NC):
        nodes = core_nodes[c]
        order = np.argsort(-indeg[nodes], kind="stable")
        wload = np.zeros(NW, np.int64)
        wcount = np.zeros(NW, np.int64)
        slot_of = np.empty(len(nodes), np.int64)
        # greedy: put next-heaviest node into least-loaded non-full window
        for i in order:
            open_w = np.nonzero(wcount < 128)[0]
            w = open_w[np.argmin(wload[open_w])]
            slot_of[i] = w * 128 + wcount[w]
            wcount[w] += 1
            wload[w] += indeg[nodes[i]]
        local_row[nodes] = slot_of

    # --- edge lists (NO self-loops: handled by identity matmul on-device),
    # bucketed by (target core, seg, window)
    e_core = node_core[tgt]
    e_lrow_t = local_row[tgt]
    e_w = e_lrow_t // 128
    e_tshift = e_lrow_t % 128
    s_core = node_core[src]
    s_lrow = local_row[src]
    e_seg = s_lrow // cfg.segrows
    # index into segment table: rank-major layout (AllGather concat on rows)
    e_idx16 = s_core * cfg.segrows + (s_lrow - e_seg * cfg.segrows)
    assert e_idx16.max() < 32768

    # counts per (core, seg, window)
    cnt = np.zeros((NC, NSEG, NW), np.int64)
    np.add.at(cnt, (e_core, e_seg, e_w), 1)
    C = np.maximum(0, -(-cnt.max(axis=0) // 128))     # [NSEG, NW]

    meta = Meta()
    meta.C = C

    # global chunk order: range -> seg -> window -> chunks
    chunk_order = []      # list of (seg, window)-chunk tuples in order
    batch_base = {}
    for (w0, w1) in cfg.ranges:
        for q in range(NSEG):
            base = len(chunk_order)
            for w in range(w0, w1):
                for _ in range(C[q, w]):
                    chunk_order.append((q, w))
            batch_base[(w0, q)] = (base, len(chunk_order) - base)
    meta.batch_chunk_base = batch_base
    meta.tot_chunks = len(chunk_order)
    meta.cbmax = max((n for (_, n) in batch_base.values()), default=1)
    TOTC = meta.tot_chunks
    TOTS = TOTC * 128

    # slot ranges per (seg,window) in the global order
    slot_base = {}
    pos = 0
    for (w0, w1) in cfg.ranges:
        for q in range(NSEG):
            for w in range(w0, w1):
                slot_base[(q, w)] = pos
                pos += C[q, w] * 128
    assert pos == TOTS

    # --- per-core edge slot arrays
    dt_bf16 = ml_dtypes.bfloat16
    dt_fp8 = ml_dtypes.float8_e4m3
    in_maps = []
    W1b = np.asarray(W1, np.float32).astype(dt_bf16)
    W2b = np.asarray(W2, np.float32).astype(dt_bf16)
    Wpb = np.asarray(Wp, np.float32).astype(dt_bf16)
    b1f = np.asarray(b1, np.float32).reshape(1, -1)
    b2f = np.asarray(b2, np.float32).reshape(1, -1)
    bpf = np.asarray(bp, np.float32).reshape(1, -1)
    b1bc = np.tile(b1f, (128, 1))                     # [128, HID]
    b2bc = np.tile(b2f, (128, 1))
    x = np.asarray(x, np.float32)

    iota_rep = np.tile(np.arange(128, dtype=np.int8)[None, :],
                       (128, max(meta.cbmax, 1)))

    for c in range(NC):
        mask = e_core == c
        cs, cw, ct, cq = (e_seg[mask], e_w[mask], e_tshift[mask],
                          e_idx16[mask])
        idx_flat = np.zeros(TOTS, np.int64)
        tsh_flat = np.full(TOTS, -1, np.int64)
        # fill per (q,w)
        key = cs * NW + cw
        order = np.argsort(key, kind="stable")
        ks, kt, kq, kw = cq[order], ct[order], cs[order], cw[order]
        # walk groups
        uniq, starts = np.unique(kq * NW + kw, return_index=True)
        starts = list(starts) + [len(ks)]
        for u, s0, s1 in zip(uniq, starts[:-1], starts[1:]):
            q, w = int(u) // NW, int(u) % NW
            n = s1 - s0
            b = slot_base[(q, w)]
            assert n <= C[q, w] * 128
            so = np.argsort(ks[s0:s1], kind="stable")
            idx_flat[b:b + n] = ks[s0:s1][so]
            tsh_flat[b:b + n] = kt[s0:s1][so]

        # pack idx: [128, TOTS/16] int16, slot j -> [16r + j%16, j//16]
        idxp = idx_flat.reshape(-1, 16).T.astype(np.int16)       # [16, TOTS/16]
        idxp = np.tile(idxp, (8, 1))                              # [128, ...]
        # tshift: [128, TOTC] int8: chunk k slot p -> [p, k]
        tshp = tsh_flat.reshape(TOTC, 128).T.astype(np.int8)

        # xT bf16 [in_dim, p_local]
        nodes = core_nodes[c]
        xT = np.zeros((cfg.in_dim, cfg.p_local), np.float32)
        xT[:, local_row[nodes]] = x[nodes].T
        xTb = xT.astype(dt_bf16)

        # dinv cols [128, NW]
        dinv_l = np.zeros(cfg.p_local, np.float32)
        dinv_l[local_row[nodes]] = dinv[nodes]
        dinv_cols = dinv_l.reshape(NW, 128).T.copy()              # [128, NW]

        # pool selector [128, NW*gpc] bf16 and cntinv [128, khid*gpc]
        spool = np.zeros((cfg.p_local, cfg.gpc), np.float32)
        gl = batch[nodes] - c * cfg.gpc
        spool[local_row[nodes], gl] = 1.0
        spool = (spool.reshape(NW, 128, cfg.gpc).transpose(1, 0, 2)
                 .reshape(128, NW * cfg.gpc)).astype(dt_bf16)
        cnt_g = np.bincount(gl, minlength=cfg.gpc).astype(np.float32)
        cntinv = (1.0 / np.maximum(cnt_g, 1.0)).astype(np.float32)
        cntinv_rep = np.tile(cntinv[None, :], (128, cfg.khid))    # [128, khid*gpc]

        ident = np.eye(128, dtype=dt_bf16)
        identq = np.eye(128, dtype=dt_fp8)

        in_maps.append(dict(
            xT=xTb, W1=W1b, W2=W2b, Wp=Wpb,
            b1b=b1bc, b2b=b2bc,
            bp8=np.tile(bpf, (cfg.gpc, 1)).astype(np.float32),
            dinv_cols=dinv_cols,
            idx=idxp, tsh=tshp, iota=iota_rep, spool=spool,
            cntinv=cntinv_rep, ident=ident, identq=identq,
        ))
    return in_maps, meta


def build_kernel(cfg: Cfg, meta: Meta, debug=False):
    NC, NW, NSEG = cfg.n_cores, cfg.nw, cfg.nseg
    HID, OUT, GPC = cfg.hid, cfg.out_dim, cfg.gpc
    KIN, KHID = cfg.kin, cfg.khid
    C = meta.C
    TOTC = meta.tot_chunks
    TOTS = TOTC * 128
    bf16, f32 = mybir.dt.bfloat16, mybir.dt.float32
    fp8 = mybir.dt.float8e4
    tabrows = NC * cfg.segrows
    WSEG = cfg.wseg

    nc = bacc.Bacc(None, target_bir_lowering=False, debug=debug,
                   num_devices=NC if NC > 1 else None,
                   num_swdge_queues=4,
                   dynamic_dma_scratch_size=32768)

    dram_in = lambda n, s, d: nc.dram_tensor(n, s, d, kind="ExternalInput")
    xT_d = dram_in("xT", [cfg.in_dim, cfg.p_local], bf16)
    W1_d = dram_in("W1", [cfg.in_dim, HID], bf16)
    W2_d = dram_in("W2", [HID, HID], bf16)
    Wp_d = dram_in("Wp", [HID, OUT], bf16)
    b1b_d = dram_in("b1b", [128, HID], f32)
    b2b_d = dram_in("b2b", [128, HID], f32)
    bp8_d = dram_in("bp8", [GPC, OUT], f32)
    dinv_d = dram_in("dinv_cols", [128, NW], f32)
    idx_d = dram_in("idx", [128, TOTS // 16], mybir.dt.int16)
    tsh_d = dram_in("tsh", [128, TOTC], mybir.dt.int8)
    iota_d = dram_in("iota", [128, 128 * meta.cbmax], mybir.dt.int8)
    spool_d = dram_in("spool", [128, NW * GPC], bf16)
    cntinv_d = dram_in("cntinv", [128, KHID * GPC], f32)
    ident_d = dram_in("ident", [128, 128], bf16)
    identq_d = dram_in("identq", [128, 128], fp8)
    out_d = nc.dram_tensor("out", [GPC, OUT], f32, kind="ExternalOutput")

    with tile.TileContext(nc) as tc:
        with (
            tc.tile_pool(name="const", bufs=1) as cpool,
            tc.tile_pool(name="xw", bufs=3) as xwpool,
            tc.tile_pool(name="gbuf", bufs=6) as gpool,
            tc.tile_pool(name="stb", bufs=4) as stpool,
            tc.tile_pool(name="flush", bufs=4) as fpool,
            tc.tile_pool(name="gall", bufs=1) as gallpool,
            tc.tile_pool(name="psw", bufs=cfg.rw + 1, space="PSUM") as pswin,
            tc.tile_pool(name="psx", bufs=2, space="PSUM") as psx,
            tc.tile_pool(name="pspool", bufs=1, space="PSUM") as pspool,
            tc.tile_pool(name="dram", bufs=1, space="DRAM") as dram,
        ):
            # ---- constants to SBUF
            W1_t = cpool.tile([128, KIN, HID], bf16)
            nc.sync.dma_start(W1_t[:], W1_d[:].rearrange("(k p) n -> p k n", p=128))
            W2_t = cpool.tile([128, KHID, HID], bf16)
            nc.sync.dma_start(W2_t[:], W2_d[:].rearrange("(k p) n -> p k n", p=128))
            Wp_t = cpool.tile([128, KHID, OUT], bf16)
            nc.sync.dma_start(Wp_t[:], Wp_d[:].rearrange("(k p) n -> p k n", p=128))
            b1b_t = cpool.tile([128, HID], f32)
            nc.sync.dma_start(b1b_t[:], b1b_d[:])
            b2b_t = cpool.tile([128, HID], f32)
            nc.sync.dma_start(b2b_t[:], b2b_d[:])
            bp8_t = cpool.tile([GPC, OUT], f32)
            nc.sync.dma_start(bp8_t[:], bp8_d[:])
            dinv_t = cpool.tile([128, NW], f32)
            nc.sync.dma_start(dinv_t[:], dinv_d[:])
            idx_t = cpool.tile([128, TOTS // 16], mybir.dt.int16)
            nc.sync.dma_start(idx_t[:], idx_d[:])
            tsh_t = cpool.tile([128, TOTC], mybir.dt.int8)
            nc.sync.dma_start(tsh_t[:], tsh_d[:])
            iota_t = cpool.tile([128, 128 * meta.cbmax], mybir.dt.int8)
            nc.sync.dma_start(iota_t[:], iota_d[:])
            spool_t = cpool.tile([128, NW * GPC], bf16)
            nc.sync.dma_start(spool_t[:], spool_d[:])
            cntinv_t = cpool.tile([128, KHID * GPC], f32)
            nc.sync.dma_start(cntinv_t[:], cntinv_d[:])
            ident_t = cpool.tile([128, 128], bf16)
            nc.sync.dma_start(ident_t[:], ident_d[:])
            identq_t = cpool.tile([128, 128], fp8)
            nc.sync.dma_start(identq_t[:], identq_d[:])
            zrow_t = cpool.tile([1, KHID * GPC], bf16)
            nc.vector.memset(zrow_t[:], 0.0)

            # ---- AG tables (DRAM), fp8 message rows (CoreSim requires a
            # single collective writer per Shared tile -> one AG per seg)
            ag_in = [[dram.tile([cfg.segrows, HID], fp8, tag=f"agin{l}{q}",
                                name=f"agin{l}{q}")
                      for q in range(NSEG)] for l in range(2)]
            ag_out = [[dram.tile([tabrows, HID], fp8, tag=f"agout{l}{q}",
                                 name=f"agout{l}{q}",
                                 addr_space="Shared" if NC > 1 else "Local")
                       for q in range(NSEG)] for l in range(2)]

            # local g per layer, SBUF-resident (self-loop source + AG staging)
            g_sb = [gallpool.tile([128, NW, HID], fp8, tag=f"gsb{l}",
                                  name=f"gsb{l}") for l in range(2)]

            xt2_t = gallpool.tile([128, KHID, cfg.p_local], bf16, tag="xt2",
                                  name="xt2")

            def stage_seg(layer, q):
                """DMA g seg q -> ag_in (scalar engine)."""
                dst = (ag_out if NC == 1 else ag_in)[layer][q]
                nc.scalar.dma_start(
                    dst[:].rearrange("(w p) n -> p w n", p=128),
                    g_sb[layer][:, q * WSEG:(q + 1) * WSEG, :])

            def launch_ag(layer, q):
                if NC == 1:
                    return
                nc.gpsimd.collective_compute(
                    "AllGather", mybir.AluOpType.bypass,
                    replica_groups=[list(range(NC))],
                    ins=[ag_in[layer][q][:].opt()],
                    outs=[ag_out[layer][q][:].opt()],
                )

            def stage_and_ag(layer, q):
                stage_seg(layer, q)
                launch_ag(layer, q)

            # ================= dense layer 1 (segment-ordered windows) ======
            for w in range(NW):
                xw = xwpool.tile([128, KIN, 128], bf16, tag="xw")
                nc.sync.dma_start(
                    xw[:],
                    xT_d[:, w * 128:(w + 1) * 128]
                    .rearrange("(k p) n -> p k n", p=128))
                ps = psx.tile([128, HID], f32, tag="psx")
                for k in range(KIN):
                    nc.tensor.matmul(ps[:], xw[:, k, :], W1_t[:, k, :],
                                     start=(k == 0), stop=(k == KIN - 1))
                # g = dinv*h, cast to fp8 table row (ACT engine, idle)
                nc.scalar.activation(g_sb[0][:, w, :], ps[:],
                                     mybir.ActivationFunctionType.Copy,
                                     scale=dinv_t[:, w:w + 1])
                if (w + 1) % WSEG == 0:
                    stage_and_ag(0, w // WSEG)

            # ================= aggregation + layer-2 dense (fused) ==========
            gq_counter = [0]   # SWDGE queue round-robin across gather calls

            def agg_layer(layer):
                """layer 0: consume ag_out[0], produce h1' -> xt2 + g2 (+AG)
                   layer 1: consume ag_out[1], produce pooled psum"""
                if layer == 1:
                    pooled = pspool.tile([128, KHID * GPC], f32)
                    # single zeroing init for the whole pooled bank: later
                    # pooling matmuls all accumulate (start would wipe the
                    # full 2KB zero region, clobbering the sibling group)
                    nc.tensor.matmul(pooled[:], ident_t[0:1, :],
                                     zrow_t[:], start=True, stop=False,
                                     skip_group_check=True)
                bias_t = b1b_t if layer == 0 else b2b_t
                # deferred AG launches (layer 0 only): seg -> emit-after range
                pending_ag = {}
                for ri, (w0, w1) in enumerate(cfg.ranges):
                    nwin = w1 - w0
                    # PSUM is bank-granular (2KB = 512 f32 cols): pack TWO
                    # windows per bank so 2.5 ranges fit in flight. The first
                    # ident matmul's start=True zeroes the whole bank; the
                    # sibling window inits with start=False onto the zeros.
                    nbank = (nwin + 1) // 2
                    pbs = [pswin.tile([128, 2, HID], f32, tag="aggpsum",
                                      name=f"aggpsum_{layer}_{w0}_{j}")
                           for j in range(nbank)]
                    # chunks remaining per bank (both windows, all segs)
                    rem = [sum(int(C[:, w].sum())
                               for w in range(w0 + 2 * j,
                                              min(w0 + 2 * j + 2, w1)))
                           for j in range(nbank)]
                    # self-loop init: psum_w = I^T @ g_w  (exact: dinv[t]*g[t]
                    # = dinv^2*h self term after flush scaling)
                    for i in range(nwin):
                        w = w0 + i
                        j, half = i // 2, i % 2
                        last_of_bank = (rem[j] == 0 and
                                        (half == 1 or w + 1 == w1))
                        nc.tensor.matmul(
                            pbs[j][:, half, :], identq_t[:],
                            g_sb[layer][:, w, :],
                            start=(half == 0), stop=last_of_bank,
                            skip_group_check=True)
                    for q in range(NSEG):
                        base, nchk = meta.batch_chunk_base[(w0, q)]
                        if nchk == 0:
                            continue
                        gb = gpool.tile([128, meta.cbmax, HID], fp8, tag="gb")
                        # SWDGE descriptor ring caps one dma_gather at ~1024
                        # indices; split into <=GMAX-chunk pieces round-robined
                        # over 4 SWDGE queues so ring-space stalls overlap
                        GMAX = 8
                        for g0 in range(0, nchk, GMAX):
                            g1 = min(g0 + GMAX, nchk)
                            nc.gpsimd.dma_gather(
                                gb[:, g0:g1, :], ag_out[layer][q][:],
                                idx_t[:, (base + g0) * 8:(base + g1) * 8],
                                num_idxs=(g1 - g0) * 128,
                                num_idxs_reg=(g1 - g0) * 128,
                                elem_size=HID,
                                queue_num=gq_counter[0] % 4)
                            gq_counter[0] += 1
                        st = stpool.tile([128, meta.cbmax * 128], fp8, tag="st")
                        nc.vector.tensor_tensor(
                            out=st[:, :nchk * 128].rearrange(
                                "p (c i) -> p c i", i=128),
                            in0=tsh_t[:, base:base + nchk].unsqueeze(2)
                                .broadcast_to([128, nchk, 128]),
                            in1=iota_t[:, :nchk * 128].rearrange(
                                "p (c i) -> p c i", i=128),
                            op=mybir.AluOpType.is_equal)
                        ci = 0
                        for w in range(w0, w1):
                            j, half = (w - w0) // 2, (w - w0) % 2
                            for _ in range(C[q, w]):
                                rem[j] -= 1
                                nc.tensor.matmul(
                                    pbs[j][:, half, :],
                                    st[:, ci * 128:(ci + 1) * 128],
                                    gb[:, ci, :],
                                    start=False, stop=(rem[j] == 0),
                                    skip_group_check=True)
                                ci += 1
                        assert ci == nchk
                    # flush windows of this range
                    for i, w in enumerate(range(w0, w1)):
                        # t1 = dinv*psum on ACT (Copy w/ per-partition scale;
                        # Copy-only keeps the ACT table warm)
                        tmp = fpool.tile([128, HID], f32, tag="tmpf")
                        nc.scalar.activation(tmp[:], pbs[i // 2][:, i % 2, :],
                                             mybir.ActivationFunctionType.Copy,
                                             scale=dinv_t[:, w:w + 1])
                        hp0 = fpool.tile([128, HID], bf16, tag="hpre")
                        nc.vector.tensor_tensor(out=hp0[:], in0=tmp[:],
                                                in1=bias_t[:],
                                                op=mybir.AluOpType.add)
                        # relu on ACT: Relu is a filler function in every
                        # table set, so no ACT_TABLE_LOAD thrash vs Copy
                        hp = fpool.tile([128, HID], bf16, tag="hflush")
                        nc.scalar.activation(hp[:], hp0[:],
                                             mybir.ActivationFunctionType.Relu)
                        if layer == 0:
                            # transpose into xt2; dense2; g2 write
                            for h in range(KHID):
                                pt = psx.tile([128, 128], bf16, tag="psx")
                                nc.tensor.transpose(
                                    pt[:], hp[:, h * 128:(h + 1) * 128],
                                    ident_t[:])
                                nc.vector.tensor_copy(
                                    xt2_t[:, h, w * 128:(w + 1) * 128], pt[:])
                            ps2 = psx.tile([128, HID], f32, tag="psx")
                            for k in range(KHID):
                                nc.tensor.matmul(
                                    ps2[:], xt2_t[:, k, w * 128:(w + 1) * 128],
                                    W2_t[:, k, :],
                                    start=(k == 0), stop=(k == KHID - 1))
                            nc.scalar.activation(
                                g_sb[1][:, w, :], ps2[:],
                                mybir.ActivationFunctionType.Copy,
                                scale=dinv_t[:, w:w + 1])
                            if (w + 1) % WSEG == 0:
                                # stage now (ACT reaches it right after this
                                # range's flush ops); defer the collective ~2
                                # ranges so it doesn't head-of-line block
                                # pending gather calls on the Pool queue
                                stage_seg(1, w // WSEG)
                                pending_ag[w // WSEG] = ri + 2
                        else:
                            for h in range(KHID):
                                nc.tensor.matmul(
                                    pooled[:, h * GPC:(h + 1) * GPC],
                                    hp[:, h * 128:(h + 1) * 128],
                                    spool_t[:, w * GPC:(w + 1) * GPC],
                                    start=False,
                                    stop=(w == NW - 1 and h == KHID - 1),
                                    skip_group_check=True)
                    # emit any AG whose deferral window has elapsed
                    for q in list(pending_ag):
                        if ri >= pending_ag[q] or ri == len(cfg.ranges) - 1:
                            launch_ag(1, q)
                            del pending_ag[q]
                if layer == 1:
                    return pooled

            agg_layer(0)
            pooled = agg_layer(1)
            # ============= pooled -> mean -> final linear ===============
            pooledT = fpool.tile([128, KHID * GPC], bf16, tag="pooledT")
            nc.vector.tensor_tensor(out=pooledT[:], in0=pooled[:],
                                    in1=cntinv_t[:],
                                    op=mybir.AluOpType.mult)
            ps_out = psx.tile([GPC, OUT], f32, tag="psx")
            for k in range(KHID):
                nc.tensor.matmul(ps_out[:],
                                 pooledT[:, k * GPC:(k + 1) * GPC],
                                 Wp_t[:, k, :],
                                 start=(k == 0), stop=(k == KHID - 1))
            out_sb = fpool.tile([GPC, OUT], f32, tag="outsb")
            nc.vector.tensor_tensor(out=out_sb[:], in0=ps_out[:],
                                    in1=bp8_t[:], op=mybir.AluOpType.add)
            nc.sync.dma_start(out_d[:], out_sb[:])

    nc.compile()
    return nc


def kernel(**inputs) -> "np.ndarray":
    """Full-input entry point: shards the graph across 8 NeuronCores,
    runs the Bass GNN kernel, returns the full (64, 128) float32 output."""
    cfg = Cfg()
    in_maps, meta = host_prep(
        inputs["x"], inputs["edge_index"], inputs["batch"],
        inputs["W1"], inputs["b1"], inputs["W2"], inputs["b2"],
        inputs["Wp"], inputs["bp"], cfg)
    nc = build_kernel(cfg, meta, debug=False)
    from concourse.bass_utils import run_bass_kernel_spmd
    res = run_bass_kernel_spmd(nc, in_maps,
                               core_ids=list(range(cfg.n_cores)), trace=False)
    out = np.concatenate([r["out"] for r in res.results], axis=0)
    return np.ascontiguousarray(out.astype(np.float32))
